# revision 35
# baseline (speedup 1.0000x reference)
"""Causal attention (B=2, S=2048, HID=2048, H=16, D=128) on 8 NeuronCores.

Sharding: tensor-parallel over heads — core c owns heads (2c, 2c+1).
Each core: projects Q/K/V for its heads (fp16 matmuls, fp32 PSUM accum),
applies rope (sign-permutation matmul + cos/sin DVE combine), computes
causal attention with transposed scores (k on partitions, q on free) so
softmax needs no transposes, then computes its partial contribution to
the output projection (contraction over its 256 hid columns of wo).
Host sums the 8 partial outputs.

Layouts (per core):
  xT   [2048 hid, 4096 (b*2048+s)] fp16   — x transposed, replicated
  wqT/wkT/wvT [2048 hid, 256 d] fp16      — weight slices, pre-transposed
  woT  [256 hid_c, 2048 e] fp16
  cose/sine [128 d, 2048 s] fp32          — rope tables (d row = d//2 freq)
  psgn [128, 128] fp16                    — rope pair-swap sign matrix (lhsT)
  tri  [128, 128] fp16                    — causal mask for diagonal tiles
Softmax uses exp without max subtraction: scores are ~N(0,1) after the
1/sqrt(D) scale (max |score| ~ 7), safe in fp32/fp16.
"""
import math
import sys
import types

import numpy as np

B, S, HID, H = 2, 2048, 2048, 16
D = 128
NCORES = 8
HPC = H // NCORES            # heads per core
DC = HPC * D                 # hid slice per core
SC = 512                     # seq chunk
NQC = S // SC                # chunks per batch
NHT = HID // 128             # hid tiles
F16 = np.float16


def _register_ntff_hook():
    """The agent image's antenv lacks axon_hooks; recreate it so
    run_bass_kernel_spmd(trace=True) can capture NTFF profiles."""
    try:
        from antenv.axon_hooks import get_axon_ntff_profile_hook  # noqa: F401
        return
    except ImportError:
        pass
    try:
        import antenv
        from trn_agent_boot.trn_boot import _ntff_profile_via_ctypes
        mod = types.ModuleType("antenv.axon_hooks")
        _hook = [None]
        mod.set_axon_ntff_profile_hook = lambda h: _hook.__setitem__(0, h)
        mod.get_axon_ntff_profile_hook = lambda: _hook[0]
        sys.modules["antenv.axon_hooks"] = mod
        antenv.axon_hooks = mod
        mod.set_axon_ntff_profile_hook(
            _ntff_profile_via_ctypes("/opt/axon/libaxon_pjrt.so"))
    except Exception:
        pass


_CACHE = {}
LAST_RESULT = None


def _build():
    import concourse.bass as bass  # noqa: F401
    import concourse.mybir as mybir
    import concourse.tile as tile
    from concourse import bacc

    f16 = mybir.dt.float16
    f32 = mybir.dt.float32
    EXP = mybir.ActivationFunctionType.Exp
    ISCALE = float(1.0 / math.sqrt(D))

    nc = bacc.Bacc("TRN2", target_bir_lowering=False, debug=False,
                   num_devices=NCORES)

    xT = nc.dram_tensor("xT", [HID, B * S], f16, kind="ExternalInput").ap()
    wqT = nc.dram_tensor("wqT", [HID, DC], f16, kind="ExternalInput").ap()
    wkT = nc.dram_tensor("wkT", [HID, DC], f16, kind="ExternalInput").ap()
    wvT = nc.dram_tensor("wvT", [HID, DC], f16, kind="ExternalInput").ap()
    woT = nc.dram_tensor("woT", [DC, HID], f16, kind="ExternalInput").ap()
    cose = nc.dram_tensor("cose", [128, S], f32, kind="ExternalInput").ap()
    sine = nc.dram_tensor("sine", [128, S], f32, kind="ExternalInput").ap()
    psgn = nc.dram_tensor("psgn", [128, 128], f16, kind="ExternalInput").ap()
    tri = nc.dram_tensor("tri", [128, 128], f16, kind="ExternalInput").ap()
    ident = nc.dram_tensor("ident", [128, 128], f16, kind="ExternalInput").ap()
    out = nc.dram_tensor("out", [B * S, HID], f32, kind="ExternalOutput").ap()

    with tile.TileContext(nc) as tc:
        with tc.tile_pool(name="consts", bufs=1) as consts, \
             tc.tile_pool(name="kv", bufs=1) as kv, \
             tc.tile_pool(name="xtp", bufs=2) as xtp, \
             tc.tile_pool(name="qdp", bufs=2) as qdp, \
             tc.tile_pool(name="ropep", bufs=4) as ropep, \
             tc.tile_pool(name="ptp", bufs=4) as ptp, \
             tc.tile_pool(name="onp", bufs=2) as onp, \
             tc.tile_pool(name="bcp", bufs=2) as bcp, \
             tc.tile_pool(name="recp", bufs=2) as recp, \
             tc.tile_pool(name="finp", bufs=8) as finp, \
             tc.tile_pool(name="pstream", bufs=2, space="PSUM") as pstream, \
             tc.tile_pool(name="pacc", bufs=1, space="PSUM") as pacc, \
             tc.tile_pool(name="pproj", bufs=4, space="PSUM") as pproj, \
             tc.tile_pool(name="lps", bufs=1, space="PSUM") as lps:

            # ---- constants (each loaded with a single DMA) ----
            # order matters: small psgn/tri first so PE warmup can start,
            # then the weights needed by the first chunk; cos/sin/wo are
            # not needed until ~20us in.
            psgn_sb = consts.tile([128, 128], f16)
            tri_sb = consts.tile([128, 128], f16)
            ident_sb = consts.tile([128, 128], f16)
            nc.sync.dma_start(out=psgn_sb, in_=psgn[:, :])
            nc.sync.dma_start(out=tri_sb, in_=tri[:, :])
            nc.sync.dma_start(out=ident_sb, in_=ident[:, :])
            # warm up the PE (HAM clock gate) while the big weight DMAs land
            warm_ps = pstream.tile([128, 128], f32, tag="st")
            for _ in range(48):
                nc.tensor.matmul(warm_ps, lhsT=psgn_sb, rhs=tri_sb,
                                 start=True, stop=True)
            wq_sb = consts.tile([128, NHT, DC], f16)
            wk_sb = consts.tile([128, NHT, DC], f16)
            wv_sb = consts.tile([128, NHT, DC], f16)
            for w_sb, w_ap in ((wq_sb, wqT), (wk_sb, wkT), (wv_sb, wvT)):
                nc.sync.dma_start(
                    out=w_sb,
                    in_=w_ap.rearrange("(ht p) d -> p ht d", p=128))
            cos_sb = consts.tile([128, S], f32)
            sin_sb = consts.tile([128, S], f32)
            ones_mat = consts.tile([128, 128], f16)
            nc.vector.memset(ones_mat, 1.0)
            wo_sb = consts.tile([128, HPC, HID], f16)

            def load_late_consts():
                # issued after the first chunk's xT slice so the first
                # projection matmuls aren't queued behind these transfers
                nc.sync.dma_start(out=cos_sb, in_=cose[:, :])
                nc.sync.dma_start(out=sin_sb, in_=sine[:, :])
                nc.sync.dma_start(
                    out=wo_sb, in_=woT.rearrange("(dt p) e -> p dt e", p=128))

            # ---- persistent K/V for the core's heads ----
            kd_sb = kv.tile([128, HPC, B * S], f16)     # [d, head, b*2048+s]
            v_sb = kv.tile([128, B * S // 128, DC], f16)  # [s%128, stile, (h,d)]

            def wo_phase(on_sb, s0g, sts=(0, 1, 2, 3)):
                # ---------- output projection for a finished chunk ----------
                # deep fin buffering; sub-phases are interleaved into the
                # next chunk's attention so the serial DVE fin-copy drain
                # overlaps the k-loops instead of stalling the projection
                for st in sts:
                    for ec in range(HID // 512):
                        fin_ps = pproj.tile([128, 512], f32, tag="pj")
                        for dt in range(HPC):
                            nc.tensor.matmul(
                                fin_ps,
                                lhsT=on_sb[:, dt, st * 128:(st + 1) * 128],
                                rhs=wo_sb[:, dt, ec * 512:(ec + 1) * 512],
                                start=(dt == 0), stop=(dt == HPC - 1))
                        fin_sb = finp.tile([128, 512], f32, tag="fin")
                        nc.vector.tensor_copy(out=fin_sb, in_=fin_ps)
                        nc.sync.dma_start(
                            out=out[s0g + st * 128:s0g + (st + 1) * 128,
                                    ec * 512:(ec + 1) * 512],
                            in_=fin_sb)

            pending_wo = None  # (on_sb, s0g) of the previous chunk
            for b in range(B):
                for qc in range(NQC):
                    s0g = b * S + qc * SC
                    q0 = qc * SC

                    # ---------- projection of this chunk ----------
                    xt = xtp.tile([128, NHT, SC], f16, tag="xt")
                    if s0g == 0:
                        # first chunk: split across both DMA queues so the
                        # first projection matmuls start sooner
                        half = NHT // 2
                        nc.gpsimd.dma_start(
                            out=xt[:, :half, :],
                            in_=xT[:half * 128, s0g:s0g + SC].rearrange(
                                "(ht p) s -> p ht s", p=128))
                        nc.sync.dma_start(
                            out=xt[:, half:, :],
                            in_=xT[half * 128:, s0g:s0g + SC].rearrange(
                                "(ht p) s -> p ht s", p=128))
                        load_late_consts()
                    else:
                        nc.gpsimd.dma_start(
                            out=xt,
                            in_=xT[:, s0g:s0g + SC].rearrange(
                                "(ht p) s -> p ht s", p=128))

                    qd_c = qdp.tile([128, HPC, SC], f16, tag="qd")

                    def emit_rope(acc, qraw, dest):
                        # dest = acc*cos + (psgn.T@acc)*sin — the u matmul
                        # is emitted one projection group late so the PE
                        # doesn't stall on the qraw ACT copy
                        u_ps = pstream.tile([128, SC], f32, tag="st")
                        nc.tensor.matmul(u_ps, lhsT=psgn_sb, rhs=qraw,
                                         start=True, stop=True)
                        t0 = ropep.tile([128, SC], f16, tag="t0")
                        nc.vector.tensor_mul(t0, acc, cos_sb[:, q0:q0 + SC])
                        t1 = ropep.tile([128, SC], f16, tag="t1")
                        nc.vector.tensor_mul(t1, u_ps, sin_sb[:, q0:q0 + SC])
                        nc.vector.tensor_add(dest, t0, t1)

                    rope_pending = None
                    for which in ("q", "k"):
                        w_sb = wq_sb if which == "q" else wk_sb
                        for dt in range(HPC):
                            acc = pproj.tile([128, SC], f32, tag="pj")
                            for ht in range(NHT):
                                nc.tensor.matmul(
                                    acc,
                                    lhsT=w_sb[:, ht, dt * 128:(dt + 1) * 128],
                                    rhs=xt[:, ht, :],
                                    start=(ht == 0), stop=(ht == NHT - 1))
                            qraw = ropep.tile([128, SC], f16, tag="qraw")
                            nc.scalar.copy(out=qraw, in_=acc)
                            if rope_pending is not None:
                                emit_rope(*rope_pending)
                            if which == "q":
                                dest = qd_c[:, dt, :]
                            else:
                                dest = kd_sb[:, dt, s0g:s0g + SC]
                            rope_pending = (acc, qraw, dest)
                    for st in range(SC // 128):
                        vacc = pproj.tile([128, DC], f32, tag="pj")
                        for ht in range(NHT):
                            nc.tensor.matmul(
                                vacc,
                                lhsT=xt[:, ht, st * 128:(st + 1) * 128],
                                rhs=wv_sb[:, ht, :],
                                start=(ht == 0), stop=(ht == NHT - 1))
                        if rope_pending is not None:
                            emit_rope(*rope_pending)
                            rope_pending = None
                        nc.vector.tensor_copy(
                            out=v_sb[:, (s0g // 128) + st, :], in_=vacc)

                    # previous chunk's output projection: half here (PE work
                    # covering the rope DVE chain), half between the heads
                    # (so the fin-copy drain overlaps the h1 k-loop)
                    if pending_wo is not None:
                        wo_phase(*pending_wo, sts=(0, 1))

                    # ---------- attention for this chunk ----------
                    on_sb = onp.tile([128, HPC, SC], f16, tag="on")
                    for h in range(HPC):
                        oT = pacc.tile([128, SC], f32, tag="acc")
                        # column sums of probs, broadcast to all 128
                        # partitions by an all-ones stationary matrix
                        lbc_ps = lps.tile([128, SC], f32, tag="l")
                        nkt = qc * 4 + 4

                        def emit_probs(kt):
                            # scores matmul + exp; on the diagonal tile a
                            # second tiny matmul accumulates a -60000
                            # upper-triangle bias (tri_sb.T @ I) so exp
                            # gives exact causal zeros — no vector-engine
                            # masking in the dependency chain
                            col0 = max(0, 128 * kt - q0)
                            diag = kt >= qc * 4
                            sp = pstream.tile([128, SC], f32, tag="st")
                            nc.tensor.matmul(
                                sp[:, col0:],
                                lhsT=kd_sb[:, h,
                                           b * S + kt * 128:
                                           b * S + (kt + 1) * 128],
                                rhs=qd_c[:, h, col0:],
                                start=True, stop=not diag)
                            if diag:
                                j = 128 * (kt - qc * 4)
                                nc.tensor.matmul(
                                    sp[:, j:j + 128], lhsT=tri_sb,
                                    rhs=ident_sb, start=False, stop=True)
                            pT = ptp.tile([128, SC], f16, tag="pt")
                            nc.scalar.activation(out=pT[:, col0:],
                                                 in_=sp[:, col0:],
                                                 func=EXP, scale=ISCALE)
                            return pT

                        # software-pipelined (depth 2): scores for kt+1/kt+2
                        # are emitted before the l/PV matmuls of kt, so the
                        # PE has work while the exp for kt runs on ACT
                        pts = [emit_probs(kt) for kt in range(min(2, nkt))]
                        for kt in range(nkt):
                            col0 = max(0, 128 * kt - q0)
                            if kt + 2 < nkt:
                                pts.append(emit_probs(kt + 2))
                            pT_cur = pts.pop(0)
                            nc.tensor.matmul(
                                lbc_ps[:, col0:], lhsT=ones_mat,
                                rhs=pT_cur[:, col0:],
                                start=(kt == 0), stop=(kt == nkt - 1))
                            nc.tensor.matmul(
                                oT[:, col0:],
                                lhsT=v_sb[:, b * (S // 128) + kt,
                                          h * 128:(h + 1) * 128],
                                rhs=pT_cur[:, col0:],
                                start=(kt == 0), stop=(kt == nkt - 1))
                        # free the l PSUM bank promptly via an ACT copy
                        # (the DVE queue may be clogged by fin copies),
                        # then reciprocal + normalize from SBUF
                        l_sb = bcp.tile([128, SC], f32, tag="lsb")
                        nc.scalar.copy(out=l_sb, in_=lbc_ps)
                        rbc = bcp.tile([128, SC], f32, tag="rbc")
                        nc.vector.reciprocal(rbc, l_sb)
                        nc.vector.tensor_mul(on_sb[:, h, :], oT, rbc)
                        if h == 0 and pending_wo is not None:
                            wo_phase(*pending_wo, sts=(2, 3))
                            pending_wo = None

                    pending_wo = (on_sb, s0g)
            wo_phase(*pending_wo)
    nc.compile()
    return nc


def _prep_inputs(x, freqs_cis, wq, wk, wv, wo):
    x = np.asarray(x, dtype=np.float32)
    freqs = np.asarray(freqs_cis, dtype=np.float32)
    wq = np.asarray(wq, dtype=np.float32)
    wk = np.asarray(wk, dtype=np.float32)
    wv = np.asarray(wv, dtype=np.float32)
    wo = np.asarray(wo, dtype=np.float32)

    xT = x.reshape(B * S, HID).T.astype(F16, order="C")
    cos_e = np.ascontiguousarray(np.repeat(freqs[:, :, 0].T, 2, axis=0),
                                 dtype=np.float32)
    sin_e = np.ascontiguousarray(np.repeat(freqs[:, :, 1].T, 2, axis=0),
                                 dtype=np.float32)
    psgn = np.zeros((128, 128), F16)
    idx = np.arange(64)
    psgn[2 * idx, 2 * idx + 1] = 1.0
    psgn[2 * idx + 1, 2 * idx] = -1.0
    # causal bias, passed pre-transposed for use as matmul lhsT:
    # bias[kp, qf] = -60000 where kp > qf (future key), else 0
    kp = np.arange(128)[:, None]
    qf = np.arange(128)[None, :]
    tri = np.ascontiguousarray(np.where(kp > qf, -60000.0, 0.0).T
                               ).astype(F16)
    ident = np.eye(128, dtype=F16)

    in_maps = []
    for c in range(NCORES):
        sl = slice(DC * c, DC * (c + 1))
        in_maps.append({
            "xT": xT,
            "wqT": wq[sl, :].T.astype(F16, order="C"),
            "wkT": wk[sl, :].T.astype(F16, order="C"),
            "wvT": wv[sl, :].T.astype(F16, order="C"),
            "woT": wo[:, sl].T.astype(F16, order="C"),
            "cose": cos_e,
            "sine": sin_e,
            "psgn": psgn,
            "tri": tri,
            "ident": ident,
        })
    return in_maps


def kernel(x, freqs_cis, wq, wk, wv, wo):
    global LAST_RESULT
    _register_ntff_hook()
    from concourse import bass_utils

    if "nc" not in _CACHE:
        _CACHE["nc"] = _build()
    nc = _CACHE["nc"]

    in_maps = _prep_inputs(x, freqs_cis, wq, wk, wv, wo)
    res = bass_utils.run_bass_kernel_spmd(
        nc, in_maps, core_ids=list(range(NCORES)))
    LAST_RESULT = res
    acc = np.zeros((B * S, HID), np.float64)
    for r in res.results:
        acc += r["out"].astype(np.float64)
    return acc.reshape(B, S, HID).astype(np.float32)


# revision 36
# speedup vs baseline: 1.0100x; 1.0100x over previous
"""Causal attention (B=2, S=2048, HID=2048, H=16, D=128) on 8 NeuronCores.

Sharding: tensor-parallel over heads — core c owns heads (2c, 2c+1).
Each core: projects Q/K/V for its heads (fp16 matmuls, fp32 PSUM accum),
applies rope (sign-permutation matmul + cos/sin DVE combine), computes
causal attention with transposed scores (k on partitions, q on free) so
softmax needs no transposes, then computes its partial contribution to
the output projection (contraction over its 256 hid columns of wo).
Host sums the 8 partial outputs.

Layouts (per core):
  xT   [2048 hid, 4096 (b*2048+s)] fp16   — x transposed, replicated
  wqT/wkT/wvT [2048 hid, 256 d] fp16      — weight slices, pre-transposed
  woT  [256 hid_c, 2048 e] fp16
  cose/sine [128 d, 2048 s] fp32          — rope tables (d row = d//2 freq)
  psgn [128, 128] fp16                    — rope pair-swap sign matrix (lhsT)
  tri  [128, 128] fp16                    — causal mask for diagonal tiles
Softmax uses exp without max subtraction: scores are ~N(0,1) after the
1/sqrt(D) scale (max |score| ~ 7), safe in fp32/fp16.
"""
import math
import sys
import types

import numpy as np

B, S, HID, H = 2, 2048, 2048, 16
D = 128
NCORES = 8
HPC = H // NCORES            # heads per core
DC = HPC * D                 # hid slice per core
SC = 512                     # seq chunk
NQC = S // SC                # chunks per batch
NHT = HID // 128             # hid tiles
F16 = np.float16


def _register_ntff_hook():
    """The agent image's antenv lacks axon_hooks; recreate it so
    run_bass_kernel_spmd(trace=True) can capture NTFF profiles."""
    try:
        from antenv.axon_hooks import get_axon_ntff_profile_hook  # noqa: F401
        return
    except ImportError:
        pass
    try:
        import antenv
        from trn_agent_boot.trn_boot import _ntff_profile_via_ctypes
        mod = types.ModuleType("antenv.axon_hooks")
        _hook = [None]
        mod.set_axon_ntff_profile_hook = lambda h: _hook.__setitem__(0, h)
        mod.get_axon_ntff_profile_hook = lambda: _hook[0]
        sys.modules["antenv.axon_hooks"] = mod
        antenv.axon_hooks = mod
        mod.set_axon_ntff_profile_hook(
            _ntff_profile_via_ctypes("/opt/axon/libaxon_pjrt.so"))
    except Exception:
        pass


_CACHE = {}
LAST_RESULT = None


def _build():
    import concourse.bass as bass  # noqa: F401
    import concourse.mybir as mybir
    import concourse.tile as tile
    from concourse import bacc

    f16 = mybir.dt.float16
    f32 = mybir.dt.float32
    EXP = mybir.ActivationFunctionType.Exp
    ISCALE = float(1.0 / math.sqrt(D))

    nc = bacc.Bacc("TRN2", target_bir_lowering=False, debug=False,
                   num_devices=NCORES)

    xT = nc.dram_tensor("xT", [HID, B * S], f16, kind="ExternalInput").ap()
    wqT = nc.dram_tensor("wqT", [HID, DC], f16, kind="ExternalInput").ap()
    wkT = nc.dram_tensor("wkT", [HID, DC], f16, kind="ExternalInput").ap()
    wvT = nc.dram_tensor("wvT", [HID, DC], f16, kind="ExternalInput").ap()
    woT = nc.dram_tensor("woT", [DC, HID], f16, kind="ExternalInput").ap()
    cose = nc.dram_tensor("cose", [128, S], f32, kind="ExternalInput").ap()
    sine = nc.dram_tensor("sine", [128, S], f32, kind="ExternalInput").ap()
    psgn = nc.dram_tensor("psgn", [128, 128], f16, kind="ExternalInput").ap()
    tri = nc.dram_tensor("tri", [128, 128], f16, kind="ExternalInput").ap()
    ident = nc.dram_tensor("ident", [128, 128], f16, kind="ExternalInput").ap()
    out = nc.dram_tensor("out", [B * S, HID], f32, kind="ExternalOutput").ap()

    with tile.TileContext(nc) as tc:
        with tc.tile_pool(name="consts", bufs=1) as consts, \
             tc.tile_pool(name="kv", bufs=1) as kv, \
             tc.tile_pool(name="xtp", bufs=2) as xtp, \
             tc.tile_pool(name="qdp", bufs=2) as qdp, \
             tc.tile_pool(name="ropep", bufs=4) as ropep, \
             tc.tile_pool(name="ptp", bufs=4) as ptp, \
             tc.tile_pool(name="onp", bufs=2) as onp, \
             tc.tile_pool(name="bcp", bufs=2) as bcp, \
             tc.tile_pool(name="recp", bufs=2) as recp, \
             tc.tile_pool(name="finp", bufs=8) as finp, \
             tc.tile_pool(name="pstream", bufs=2, space="PSUM") as pstream, \
             tc.tile_pool(name="pacc", bufs=2, space="PSUM") as pacc, \
             tc.tile_pool(name="pproj", bufs=3, space="PSUM") as pproj, \
             tc.tile_pool(name="lps", bufs=1, space="PSUM") as lps:

            # ---- constants (each loaded with a single DMA) ----
            # order matters: small psgn/tri first so PE warmup can start,
            # then the weights needed by the first chunk; cos/sin/wo are
            # not needed until ~20us in.
            psgn_sb = consts.tile([128, 128], f16)
            tri_sb = consts.tile([128, 128], f16)
            ident_sb = consts.tile([128, 128], f16)
            nc.sync.dma_start(out=psgn_sb, in_=psgn[:, :])
            nc.sync.dma_start(out=tri_sb, in_=tri[:, :])
            nc.sync.dma_start(out=ident_sb, in_=ident[:, :])
            # warm up the PE (HAM clock gate) while the big weight DMAs land
            warm_ps = pstream.tile([128, 128], f32, tag="st")
            for _ in range(48):
                nc.tensor.matmul(warm_ps, lhsT=psgn_sb, rhs=tri_sb,
                                 start=True, stop=True)
            wq_sb = consts.tile([128, NHT, DC], f16)
            wk_sb = consts.tile([128, NHT, DC], f16)
            wv_sb = consts.tile([128, NHT, DC], f16)
            for w_sb, w_ap in ((wq_sb, wqT), (wk_sb, wkT), (wv_sb, wvT)):
                nc.sync.dma_start(
                    out=w_sb,
                    in_=w_ap.rearrange("(ht p) d -> p ht d", p=128))
            cos_sb = consts.tile([128, S], f32)
            sin_sb = consts.tile([128, S], f32)
            ones_mat = consts.tile([128, 128], f16)
            nc.vector.memset(ones_mat, 1.0)
            wo_sb = consts.tile([128, HPC, HID], f16)

            def load_late_consts():
                # issued after the first chunk's xT slice so the first
                # projection matmuls aren't queued behind these transfers
                nc.sync.dma_start(out=cos_sb, in_=cose[:, :])
                nc.sync.dma_start(out=sin_sb, in_=sine[:, :])
                nc.sync.dma_start(
                    out=wo_sb, in_=woT.rearrange("(dt p) e -> p dt e", p=128))

            # ---- persistent K/V for the core's heads ----
            kd_sb = kv.tile([128, HPC, B * S], f16)     # [d, head, b*2048+s]
            v_sb = kv.tile([128, B * S // 128, DC], f16)  # [s%128, stile, (h,d)]

            def wo_phase(on_sb, s0g, sts=(0, 1, 2, 3)):
                # ---------- output projection for a finished chunk ----------
                # deep fin buffering; sub-phases are interleaved into the
                # next chunk's attention so the serial DVE fin-copy drain
                # overlaps the k-loops instead of stalling the projection
                for st in sts:
                    for ec in range(HID // 512):
                        fin_ps = pproj.tile([128, 512], f32, tag="pj")
                        for dt in range(HPC):
                            nc.tensor.matmul(
                                fin_ps,
                                lhsT=on_sb[:, dt, st * 128:(st + 1) * 128],
                                rhs=wo_sb[:, dt, ec * 512:(ec + 1) * 512],
                                start=(dt == 0), stop=(dt == HPC - 1))
                        fin_sb = finp.tile([128, 512], f32, tag="fin")
                        nc.vector.tensor_copy(out=fin_sb, in_=fin_ps)
                        nc.sync.dma_start(
                            out=out[s0g + st * 128:s0g + (st + 1) * 128,
                                    ec * 512:(ec + 1) * 512],
                            in_=fin_sb)

            pending_wo = None  # (on_sb, s0g) of the previous chunk
            for b in range(B):
                for qc in range(NQC):
                    s0g = b * S + qc * SC
                    q0 = qc * SC

                    # ---------- projection of this chunk ----------
                    xt = xtp.tile([128, NHT, SC], f16, tag="xt")
                    if s0g == 0:
                        # first chunk: split across both DMA queues so the
                        # first projection matmuls start sooner
                        half = NHT // 2
                        nc.gpsimd.dma_start(
                            out=xt[:, :half, :],
                            in_=xT[:half * 128, s0g:s0g + SC].rearrange(
                                "(ht p) s -> p ht s", p=128))
                        nc.sync.dma_start(
                            out=xt[:, half:, :],
                            in_=xT[half * 128:, s0g:s0g + SC].rearrange(
                                "(ht p) s -> p ht s", p=128))
                        load_late_consts()
                    else:
                        nc.gpsimd.dma_start(
                            out=xt,
                            in_=xT[:, s0g:s0g + SC].rearrange(
                                "(ht p) s -> p ht s", p=128))

                    qd_c = qdp.tile([128, HPC, SC], f16, tag="qd")

                    def emit_rope(acc, qraw, dest):
                        # dest = acc*cos + (psgn.T@acc)*sin — the u matmul
                        # is emitted one projection group late so the PE
                        # doesn't stall on the qraw ACT copy
                        u_ps = pstream.tile([128, SC], f32, tag="st")
                        nc.tensor.matmul(u_ps, lhsT=psgn_sb, rhs=qraw,
                                         start=True, stop=True)
                        t0 = ropep.tile([128, SC], f16, tag="t0")
                        nc.vector.tensor_mul(t0, acc, cos_sb[:, q0:q0 + SC])
                        t1 = ropep.tile([128, SC], f16, tag="t1")
                        nc.vector.tensor_mul(t1, u_ps, sin_sb[:, q0:q0 + SC])
                        nc.vector.tensor_add(dest, t0, t1)

                    rope_pending = None
                    for which in ("q", "k"):
                        w_sb = wq_sb if which == "q" else wk_sb
                        for dt in range(HPC):
                            acc = pproj.tile([128, SC], f32, tag="pj")
                            for ht in range(NHT):
                                nc.tensor.matmul(
                                    acc,
                                    lhsT=w_sb[:, ht, dt * 128:(dt + 1) * 128],
                                    rhs=xt[:, ht, :],
                                    start=(ht == 0), stop=(ht == NHT - 1))
                            qraw = ropep.tile([128, SC], f16, tag="qraw")
                            nc.scalar.copy(out=qraw, in_=acc)
                            if rope_pending is not None:
                                emit_rope(*rope_pending)
                            if which == "q":
                                dest = qd_c[:, dt, :]
                            else:
                                dest = kd_sb[:, dt, s0g:s0g + SC]
                            rope_pending = (acc, qraw, dest)
                    for st in range(SC // 128):
                        vacc = pproj.tile([128, DC], f32, tag="pj")
                        for ht in range(NHT):
                            nc.tensor.matmul(
                                vacc,
                                lhsT=xt[:, ht, st * 128:(st + 1) * 128],
                                rhs=wv_sb[:, ht, :],
                                start=(ht == 0), stop=(ht == NHT - 1))
                        if rope_pending is not None:
                            emit_rope(*rope_pending)
                            rope_pending = None
                        nc.vector.tensor_copy(
                            out=v_sb[:, (s0g // 128) + st, :], in_=vacc)

                    # previous chunk's output projection: half here (PE work
                    # covering the rope DVE chain), half between the heads
                    # (so the fin-copy drain overlaps the h1 k-loop)
                    if pending_wo is not None:
                        wo_phase(*pending_wo, sts=(0, 1))

                    # ---------- attention for this chunk ----------
                    on_sb = onp.tile([128, HPC, SC], f16, tag="on")
                    for h in range(HPC):
                        oT = pacc.tile([128, SC], f32, tag="acc")
                        # column sums of probs, broadcast to all 128
                        # partitions by an all-ones stationary matrix
                        lbc_ps = lps.tile([128, SC], f32, tag="l")
                        nkt = qc * 4 + 4

                        def emit_probs(kt):
                            # scores matmul + exp; on the diagonal tile a
                            # second tiny matmul accumulates a -60000
                            # upper-triangle bias (tri_sb.T @ I) so exp
                            # gives exact causal zeros — no vector-engine
                            # masking in the dependency chain
                            col0 = max(0, 128 * kt - q0)
                            diag = kt >= qc * 4
                            sp = pstream.tile([128, SC], f32, tag="st")
                            nc.tensor.matmul(
                                sp[:, col0:],
                                lhsT=kd_sb[:, h,
                                           b * S + kt * 128:
                                           b * S + (kt + 1) * 128],
                                rhs=qd_c[:, h, col0:],
                                start=True, stop=not diag)
                            if diag:
                                j = 128 * (kt - qc * 4)
                                nc.tensor.matmul(
                                    sp[:, j:j + 128], lhsT=tri_sb,
                                    rhs=ident_sb, start=False, stop=True)
                            pT = ptp.tile([128, SC], f16, tag="pt")
                            nc.scalar.activation(out=pT[:, col0:],
                                                 in_=sp[:, col0:],
                                                 func=EXP, scale=ISCALE)
                            return pT

                        # software-pipelined (depth 2): scores for kt+1/kt+2
                        # are emitted before the l/PV matmuls of kt, so the
                        # PE has work while the exp for kt runs on ACT
                        pts = [emit_probs(kt) for kt in range(min(2, nkt))]
                        for kt in range(nkt):
                            col0 = max(0, 128 * kt - q0)
                            if kt + 2 < nkt:
                                pts.append(emit_probs(kt + 2))
                            pT_cur = pts.pop(0)
                            nc.tensor.matmul(
                                lbc_ps[:, col0:], lhsT=ones_mat,
                                rhs=pT_cur[:, col0:],
                                start=(kt == 0), stop=(kt == nkt - 1))
                            nc.tensor.matmul(
                                oT[:, col0:],
                                lhsT=v_sb[:, b * (S // 128) + kt,
                                          h * 128:(h + 1) * 128],
                                rhs=pT_cur[:, col0:],
                                start=(kt == 0), stop=(kt == nkt - 1))
                        # free the l PSUM bank promptly via an ACT copy
                        # (the DVE queue may be clogged by fin copies),
                        # then reciprocal + normalize from SBUF
                        l_sb = bcp.tile([128, SC], f32, tag="lsb")
                        nc.scalar.copy(out=l_sb, in_=lbc_ps)
                        rbc = bcp.tile([128, SC], f32, tag="rbc")
                        nc.vector.reciprocal(rbc, l_sb)
                        nc.vector.tensor_mul(on_sb[:, h, :], oT, rbc)
                        if h == 0 and pending_wo is not None:
                            wo_phase(*pending_wo, sts=(2, 3))
                            pending_wo = None

                    pending_wo = (on_sb, s0g)
            wo_phase(*pending_wo)
    nc.compile()
    return nc


def _prep_inputs(x, freqs_cis, wq, wk, wv, wo):
    x = np.asarray(x, dtype=np.float32)
    freqs = np.asarray(freqs_cis, dtype=np.float32)
    wq = np.asarray(wq, dtype=np.float32)
    wk = np.asarray(wk, dtype=np.float32)
    wv = np.asarray(wv, dtype=np.float32)
    wo = np.asarray(wo, dtype=np.float32)

    xT = x.reshape(B * S, HID).T.astype(F16, order="C")
    cos_e = np.ascontiguousarray(np.repeat(freqs[:, :, 0].T, 2, axis=0),
                                 dtype=np.float32)
    sin_e = np.ascontiguousarray(np.repeat(freqs[:, :, 1].T, 2, axis=0),
                                 dtype=np.float32)
    psgn = np.zeros((128, 128), F16)
    idx = np.arange(64)
    psgn[2 * idx, 2 * idx + 1] = 1.0
    psgn[2 * idx + 1, 2 * idx] = -1.0
    # causal bias, passed pre-transposed for use as matmul lhsT:
    # bias[kp, qf] = -60000 where kp > qf (future key), else 0
    kp = np.arange(128)[:, None]
    qf = np.arange(128)[None, :]
    tri = np.ascontiguousarray(np.where(kp > qf, -60000.0, 0.0).T
                               ).astype(F16)
    ident = np.eye(128, dtype=F16)

    in_maps = []
    for c in range(NCORES):
        sl = slice(DC * c, DC * (c + 1))
        in_maps.append({
            "xT": xT,
            "wqT": wq[sl, :].T.astype(F16, order="C"),
            "wkT": wk[sl, :].T.astype(F16, order="C"),
            "wvT": wv[sl, :].T.astype(F16, order="C"),
            "woT": wo[:, sl].T.astype(F16, order="C"),
            "cose": cos_e,
            "sine": sin_e,
            "psgn": psgn,
            "tri": tri,
            "ident": ident,
        })
    return in_maps


def kernel(x, freqs_cis, wq, wk, wv, wo):
    global LAST_RESULT
    _register_ntff_hook()
    from concourse import bass_utils

    if "nc" not in _CACHE:
        _CACHE["nc"] = _build()
    nc = _CACHE["nc"]

    in_maps = _prep_inputs(x, freqs_cis, wq, wk, wv, wo)
    res = bass_utils.run_bass_kernel_spmd(
        nc, in_maps, core_ids=list(range(NCORES)))
    LAST_RESULT = res
    acc = np.zeros((B * S, HID), np.float64)
    for r in res.results:
        acc += r["out"].astype(np.float64)
    return acc.reshape(B, S, HID).astype(np.float32)


# revision 40
# speedup vs baseline: 1.0154x; 1.0054x over previous
"""Causal attention (B=2, S=2048, HID=2048, H=16, D=128) on 8 NeuronCores.

Sharding: tensor-parallel over heads — core c owns heads (2c, 2c+1).
Each core: projects Q/K/V for its heads (fp16 matmuls, fp32 PSUM accum),
applies rope (sign-permutation matmul + cos/sin DVE combine), computes
causal attention with transposed scores (k on partitions, q on free) so
softmax needs no transposes, then computes its partial contribution to
the output projection (contraction over its 256 hid columns of wo).
Host sums the 8 partial outputs.

Layouts (per core):
  xT   [2048 hid, 4096 (b*2048+s)] fp16   — x transposed, replicated
  wqT/wkT/wvT [2048 hid, 256 d] fp16      — weight slices, pre-transposed
  woT  [256 hid_c, 2048 e] fp16
  cose/sine [128 d, 2048 s] fp32          — rope tables (d row = d//2 freq)
  psgn [128, 128] fp16                    — rope pair-swap sign matrix (lhsT)
  tri  [128, 128] fp16                    — causal mask for diagonal tiles
Softmax uses exp without max subtraction: scores are ~N(0,1) after the
1/sqrt(D) scale (max |score| ~ 7), safe in fp32/fp16.
"""
import math
import sys
import types

import numpy as np

B, S, HID, H = 2, 2048, 2048, 16
D = 128
NCORES = 8
HPC = H // NCORES            # heads per core
DC = HPC * D                 # hid slice per core
SC = 512                     # seq chunk
NQC = S // SC                # chunks per batch
NHT = HID // 128             # hid tiles
F16 = np.float16


def _register_ntff_hook():
    """The agent image's antenv lacks axon_hooks; recreate it so
    run_bass_kernel_spmd(trace=True) can capture NTFF profiles."""
    try:
        from antenv.axon_hooks import get_axon_ntff_profile_hook  # noqa: F401
        return
    except ImportError:
        pass
    try:
        import antenv
        from trn_agent_boot.trn_boot import _ntff_profile_via_ctypes
        mod = types.ModuleType("antenv.axon_hooks")
        _hook = [None]
        mod.set_axon_ntff_profile_hook = lambda h: _hook.__setitem__(0, h)
        mod.get_axon_ntff_profile_hook = lambda: _hook[0]
        sys.modules["antenv.axon_hooks"] = mod
        antenv.axon_hooks = mod
        mod.set_axon_ntff_profile_hook(
            _ntff_profile_via_ctypes("/opt/axon/libaxon_pjrt.so"))
    except Exception:
        pass


_CACHE = {}
LAST_RESULT = None


def _build():
    import concourse.bass as bass  # noqa: F401
    import concourse.mybir as mybir
    import concourse.tile as tile
    from concourse import bacc

    f16 = mybir.dt.float16
    f32 = mybir.dt.float32
    EXP = mybir.ActivationFunctionType.Exp
    ISCALE = float(1.0 / math.sqrt(D))

    nc = bacc.Bacc("TRN2", target_bir_lowering=False, debug=False,
                   num_devices=NCORES)

    xT = nc.dram_tensor("xT", [HID, B * S], f16, kind="ExternalInput").ap()
    wqT = nc.dram_tensor("wqT", [HID, DC], f16, kind="ExternalInput").ap()
    wkT = nc.dram_tensor("wkT", [HID, DC], f16, kind="ExternalInput").ap()
    wvT = nc.dram_tensor("wvT", [HID, DC], f16, kind="ExternalInput").ap()
    woT = nc.dram_tensor("woT", [DC, HID], f16, kind="ExternalInput").ap()
    cose = nc.dram_tensor("cose", [128, S], f32, kind="ExternalInput").ap()
    sine = nc.dram_tensor("sine", [128, S], f32, kind="ExternalInput").ap()
    psgn = nc.dram_tensor("psgn", [128, 128], f16, kind="ExternalInput").ap()
    tri = nc.dram_tensor("tri", [128, 128], f16, kind="ExternalInput").ap()
    ident = nc.dram_tensor("ident", [128, 128], f16, kind="ExternalInput").ap()
    out = nc.dram_tensor("out", [B * S, HID], f32, kind="ExternalOutput").ap()

    with tile.TileContext(nc) as tc:
        with tc.tile_pool(name="consts", bufs=1) as consts, \
             tc.tile_pool(name="kv", bufs=1) as kv, \
             tc.tile_pool(name="xtp", bufs=2) as xtp, \
             tc.tile_pool(name="qdp", bufs=2) as qdp, \
             tc.tile_pool(name="ropep", bufs=4) as ropep, \
             tc.tile_pool(name="ptp", bufs=4) as ptp, \
             tc.tile_pool(name="onp", bufs=2) as onp, \
             tc.tile_pool(name="bcp", bufs=2) as bcp, \
             tc.tile_pool(name="finp", bufs=8) as finp, \
             tc.tile_pool(name="pstream", bufs=2, space="PSUM") as pstream, \
             tc.tile_pool(name="pacc", bufs=2, space="PSUM") as pacc, \
             tc.tile_pool(name="pproj", bufs=3, space="PSUM") as pproj, \
             tc.tile_pool(name="lps", bufs=1, space="PSUM") as lps:

            # ---- constants (each loaded with a single DMA) ----
            # order matters: small psgn/tri first so PE warmup can start,
            # then the weights needed by the first chunk; cos/sin/wo are
            # not needed until ~20us in.
            psgn_sb = consts.tile([128, 128], f16)
            tri_sb = consts.tile([128, 128], f16)
            ident_sb = consts.tile([128, 128], f16)
            nc.sync.dma_start(out=psgn_sb, in_=psgn[:, :])
            nc.sync.dma_start(out=tri_sb, in_=tri[:, :])
            nc.sync.dma_start(out=ident_sb, in_=ident[:, :])
            # warm up the PE (HAM clock gate) while the big weight DMAs land
            warm_ps = pstream.tile([128, 128], f32, tag="st")
            for _ in range(48):
                nc.tensor.matmul(warm_ps, lhsT=psgn_sb, rhs=tri_sb,
                                 start=True, stop=True)
            wq_sb = consts.tile([128, NHT, DC], f16)
            wk_sb = consts.tile([128, NHT, DC], f16)
            wv_sb = consts.tile([128, NHT, DC], f16)
            nc.sync.dma_start(
                out=wq_sb, in_=wqT.rearrange("(ht p) d -> p ht d", p=128))
            cos_sb = consts.tile([128, S], f32)
            sin_sb = consts.tile([128, S], f32)
            ones_mat = consts.tile([128, 128], f16)
            nc.vector.memset(ones_mat, 1.0)
            wo_sb = consts.tile([128, HPC, HID], f16)

            def load_late_consts():
                # issued after the first chunk's xT slice so the first
                # projection matmuls aren't queued behind these transfers;
                # ordered by first-consumption time
                nc.sync.dma_start(
                    out=wk_sb, in_=wkT.rearrange("(ht p) d -> p ht d", p=128))
                nc.sync.dma_start(
                    out=wv_sb, in_=wvT.rearrange("(ht p) d -> p ht d", p=128))
                nc.sync.dma_start(out=cos_sb, in_=cose[:, :])
                nc.sync.dma_start(out=sin_sb, in_=sine[:, :])
                nc.sync.dma_start(
                    out=wo_sb, in_=woT.rearrange("(dt p) e -> p dt e", p=128))

            # ---- persistent K/V for the core's heads ----
            kd_sb = kv.tile([128, HPC, B * S], f16)     # [d, head, b*2048+s]
            v_sb = kv.tile([128, B * S // 128, DC], f16)  # [s%128, stile, (h,d)]

            def wo_phase(on_sb, s0g, sts=(0, 1, 2, 3)):
                # ---------- output projection for a finished chunk ----------
                # deep fin buffering; sub-phases are interleaved into the
                # next chunk's attention so the serial DVE fin-copy drain
                # overlaps the k-loops instead of stalling the projection
                for st in sts:
                    for ec in range(HID // 512):
                        fin_ps = pproj.tile([128, 512], f32, tag="pj")
                        for dt in range(HPC):
                            nc.tensor.matmul(
                                fin_ps,
                                lhsT=on_sb[:, dt, st * 128:(st + 1) * 128],
                                rhs=wo_sb[:, dt, ec * 512:(ec + 1) * 512],
                                start=(dt == 0), stop=(dt == HPC - 1))
                        fin_sb = finp.tile([128, 512], f32, tag="fin")
                        nc.vector.tensor_copy(out=fin_sb, in_=fin_ps)
                        nc.sync.dma_start(
                            out=out[s0g + st * 128:s0g + (st + 1) * 128,
                                    ec * 512:(ec + 1) * 512],
                            in_=fin_sb)

            pending_wo = None  # (on_sb, s0g) of the previous chunk
            for b in range(B):
                for qc in range(NQC):
                    s0g = b * S + qc * SC
                    q0 = qc * SC

                    # ---------- projection of this chunk ----------
                    xt = xtp.tile([128, NHT, SC], f16, tag="xt")
                    if s0g == 0:
                        # first chunk: split across both DMA queues, with
                        # the first-consumed half (ht 0-7) on the sync queue
                        # right behind wq so the first matmuls start sooner
                        half = NHT // 2
                        nc.sync.dma_start(
                            out=xt[:, :half, :],
                            in_=xT[:half * 128, s0g:s0g + SC].rearrange(
                                "(ht p) s -> p ht s", p=128))
                        nc.gpsimd.dma_start(
                            out=xt[:, half:, :],
                            in_=xT[half * 128:, s0g:s0g + SC].rearrange(
                                "(ht p) s -> p ht s", p=128))
                        load_late_consts()
                    else:
                        nc.gpsimd.dma_start(
                            out=xt,
                            in_=xT[:, s0g:s0g + SC].rearrange(
                                "(ht p) s -> p ht s", p=128))

                    qd_c = qdp.tile([128, HPC, SC], f16, tag="qd")

                    def emit_rope(acc, qraw, dest):
                        # dest = acc*cos + (psgn.T@acc)*sin — the u matmul
                        # is emitted one projection group late so the PE
                        # doesn't stall on the qraw ACT copy
                        u_ps = pstream.tile([128, SC], f32, tag="st")
                        nc.tensor.matmul(u_ps, lhsT=psgn_sb, rhs=qraw,
                                         start=True, stop=True)
                        t0 = ropep.tile([128, SC], f16, tag="t0")
                        nc.vector.tensor_mul(t0, acc, cos_sb[:, q0:q0 + SC])
                        t1 = ropep.tile([128, SC], f16, tag="t1")
                        nc.vector.tensor_mul(t1, u_ps, sin_sb[:, q0:q0 + SC])
                        nc.vector.tensor_add(dest, t0, t1)

                    rope_pending = None
                    for which in ("q", "k"):
                        w_sb = wq_sb if which == "q" else wk_sb
                        for dt in range(HPC):
                            acc = pproj.tile([128, SC], f32, tag="pj")
                            for ht in range(NHT):
                                nc.tensor.matmul(
                                    acc,
                                    lhsT=w_sb[:, ht, dt * 128:(dt + 1) * 128],
                                    rhs=xt[:, ht, :],
                                    start=(ht == 0), stop=(ht == NHT - 1))
                            qraw = ropep.tile([128, SC], f16, tag="qraw")
                            nc.scalar.copy(out=qraw, in_=acc)
                            if rope_pending is not None:
                                emit_rope(*rope_pending)
                            if which == "q":
                                dest = qd_c[:, dt, :]
                            else:
                                dest = kd_sb[:, dt, s0g:s0g + SC]
                            rope_pending = (acc, qraw, dest)
                    for st in range(SC // 128):
                        vacc = pproj.tile([128, DC], f32, tag="pj")
                        for ht in range(NHT):
                            nc.tensor.matmul(
                                vacc,
                                lhsT=xt[:, ht, st * 128:(st + 1) * 128],
                                rhs=wv_sb[:, ht, :],
                                start=(ht == 0), stop=(ht == NHT - 1))
                        if rope_pending is not None:
                            emit_rope(*rope_pending)
                            rope_pending = None
                        nc.vector.tensor_copy(
                            out=v_sb[:, (s0g // 128) + st, :], in_=vacc)

                    # previous chunk's output projection: half here (PE work
                    # covering the rope DVE chain), half between the heads
                    # (so the fin-copy drain overlaps the h1 k-loop)
                    if pending_wo is not None:
                        wo_phase(*pending_wo, sts=(0, 1))

                    # ---------- attention for this chunk ----------
                    on_sb = onp.tile([128, HPC, SC], f16, tag="on")
                    for h in range(HPC):
                        oT = pacc.tile([128, SC], f32, tag="acc")
                        # column sums of probs, broadcast to all 128
                        # partitions by an all-ones stationary matrix
                        lbc_ps = lps.tile([128, SC], f32, tag="l")
                        nkt = qc * 4 + 4

                        def emit_probs(kt):
                            # scores matmul + exp; on the diagonal tile a
                            # second tiny matmul accumulates a -60000
                            # upper-triangle bias (tri_sb.T @ I) so exp
                            # gives exact causal zeros — no vector-engine
                            # masking in the dependency chain
                            col0 = max(0, 128 * kt - q0)
                            diag = kt >= qc * 4
                            sp = pstream.tile([128, SC], f32, tag="st")
                            nc.tensor.matmul(
                                sp[:, col0:],
                                lhsT=kd_sb[:, h,
                                           b * S + kt * 128:
                                           b * S + (kt + 1) * 128],
                                rhs=qd_c[:, h, col0:],
                                start=True, stop=not diag)
                            if diag:
                                j = 128 * (kt - qc * 4)
                                nc.tensor.matmul(
                                    sp[:, j:j + 128], lhsT=tri_sb,
                                    rhs=ident_sb, start=False, stop=True)
                            pT = ptp.tile([128, SC], f16, tag="pt")
                            nc.scalar.activation(out=pT[:, col0:],
                                                 in_=sp[:, col0:],
                                                 func=EXP, scale=ISCALE)
                            return pT

                        # software-pipelined (depth 2): scores for kt+1/kt+2
                        # are emitted before the l/PV matmuls of kt, so the
                        # PE has work while the exp for kt runs on ACT
                        pts = [emit_probs(kt) for kt in range(min(2, nkt))]
                        for kt in range(nkt):
                            col0 = max(0, 128 * kt - q0)
                            if kt + 2 < nkt:
                                pts.append(emit_probs(kt + 2))
                            pT_cur = pts.pop(0)
                            nc.tensor.matmul(
                                lbc_ps[:, col0:], lhsT=ones_mat,
                                rhs=pT_cur[:, col0:],
                                start=(kt == 0), stop=(kt == nkt - 1))
                            nc.tensor.matmul(
                                oT[:, col0:],
                                lhsT=v_sb[:, b * (S // 128) + kt,
                                          h * 128:(h + 1) * 128],
                                rhs=pT_cur[:, col0:],
                                start=(kt == 0), stop=(kt == nkt - 1))
                        # free the l PSUM bank promptly via an ACT copy
                        # (the DVE queue may be clogged by fin copies),
                        # then reciprocal + normalize from SBUF
                        l_sb = bcp.tile([128, SC], f32, tag="lsb")
                        nc.scalar.copy(out=l_sb, in_=lbc_ps)
                        rbc = bcp.tile([128, SC], f32, tag="rbc")
                        nc.vector.reciprocal(rbc, l_sb)
                        nc.vector.tensor_mul(on_sb[:, h, :], oT, rbc)
                        if h == 0 and pending_wo is not None:
                            wo_phase(*pending_wo, sts=(2, 3))
                            pending_wo = None

                    pending_wo = (on_sb, s0g)
            wo_phase(*pending_wo)
    nc.compile()
    return nc


def _prep_inputs(x, freqs_cis, wq, wk, wv, wo):
    x = np.asarray(x, dtype=np.float32)
    freqs = np.asarray(freqs_cis, dtype=np.float32)
    wq = np.asarray(wq, dtype=np.float32)
    wk = np.asarray(wk, dtype=np.float32)
    wv = np.asarray(wv, dtype=np.float32)
    wo = np.asarray(wo, dtype=np.float32)

    xT = x.reshape(B * S, HID).T.astype(F16, order="C")
    cos_e = np.ascontiguousarray(np.repeat(freqs[:, :, 0].T, 2, axis=0),
                                 dtype=np.float32)
    sin_e = np.ascontiguousarray(np.repeat(freqs[:, :, 1].T, 2, axis=0),
                                 dtype=np.float32)
    psgn = np.zeros((128, 128), F16)
    idx = np.arange(64)
    psgn[2 * idx, 2 * idx + 1] = 1.0
    psgn[2 * idx + 1, 2 * idx] = -1.0
    # causal bias, passed pre-transposed for use as matmul lhsT:
    # bias[kp, qf] = -60000 where kp > qf (future key), else 0
    kp = np.arange(128)[:, None]
    qf = np.arange(128)[None, :]
    tri = np.ascontiguousarray(np.where(kp > qf, -60000.0, 0.0).T
                               ).astype(F16)
    ident = np.eye(128, dtype=F16)

    in_maps = []
    for c in range(NCORES):
        sl = slice(DC * c, DC * (c + 1))
        in_maps.append({
            "xT": xT,
            "wqT": wq[sl, :].T.astype(F16, order="C"),
            "wkT": wk[sl, :].T.astype(F16, order="C"),
            "wvT": wv[sl, :].T.astype(F16, order="C"),
            "woT": wo[:, sl].T.astype(F16, order="C"),
            "cose": cos_e,
            "sine": sin_e,
            "psgn": psgn,
            "tri": tri,
            "ident": ident,
        })
    return in_maps


def kernel(x, freqs_cis, wq, wk, wv, wo):
    global LAST_RESULT
    _register_ntff_hook()
    from concourse import bass_utils

    if "nc" not in _CACHE:
        _CACHE["nc"] = _build()
    nc = _CACHE["nc"]

    in_maps = _prep_inputs(x, freqs_cis, wq, wk, wv, wo)
    res = bass_utils.run_bass_kernel_spmd(
        nc, in_maps, core_ids=list(range(NCORES)))
    LAST_RESULT = res
    acc = np.zeros((B * S, HID), np.float64)
    for r in res.results:
        acc += r["out"].astype(np.float64)
    return acc.reshape(B, S, HID).astype(np.float32)
